# revision 10
# baseline (speedup 1.0000x reference)
"""Trainium2 Bass kernel for nn_CascadedAttention (B=8, T=128, D=512, O=512).

Strategy: data-parallel over batch across 8 NeuronCores (1 batch element
per core), weights replicated. The scan recurrence runs fully on-device,
fully unrolled, with column-major (O-on-partitions) state layout.

v2 structure (vs v1 baseline):
- state = FULL pred (incl WoY): kills the hwb bias-broadcast chain into
  next step's tanh and the epilogue WoY add. WoY enters via a Pool-made
  tmp = IUoB[t-1] + WoY_bcast consumed by the final STT.
- ep = exp(state) computed on DVE from th via exp(x) = (1+t)/(1-t),
  t = tanh(x/2) (th is already computed for the attention query) ->
  removes one ACT op per step from the critical engine.
- Wa pre-scaled by 0.5 host-side so the WaS psum pair IS the e-tanh
  bias; bias read directly from PSUM (PSUM_BIAS=True) or staged via
  cheap Pool copies (False).
- ACT issue order per step: th -> e0..e3 -> eh (nothing else on ACT).

Self-contained: hardcodes all shapes; only imports the installed
concourse (bass) stack.
"""

import sys

for _p in ("/opt/trn_rl_repo", "/root/.axon_site/_ro/trn_rl_repo"):
    if _p not in sys.path:
        sys.path.append(_p)

import numpy as np

import concourse.bass as bass
import concourse.bacc as bacc
import concourse.mybir as mybir
from concourse import tile
from concourse.bass_utils import run_bass_kernel_spmd

B, T, D, O = 8, 128, 512, 512
OT = O // 128  # 4 o-tiles
DT = D // 128  # 4 d-tiles
FP32 = mybir.dt.float32
AF = mybir.ActivationFunctionType
ALU = mybir.AluOpType

# e-tanh bias straight from the WaS psum pair (bypasses the bass SBUF-only
# assert via direct InstActivation construction). False = stage the pairs
# into SBUF with Pool copies first.
PSUM_BIAS = True
REPEAT = 1


def _act_raw(nc, out, in_, func, bias=0.0, scale=1.0):
    """nc.scalar.activation minus the bias-in-SBUF assert (allows PSUM)."""
    eng = nc.scalar
    inputs = [eng.lower_ap(in_)]
    for arg in (bias, scale, 0.0):
        if isinstance(arg, bass.AP):
            inputs.append(eng.lower_ap(arg))
        else:
            inputs.append(
                mybir.ImmediateValue(dtype=mybir.dt.float32, value=float(arg))
            )
    return eng.add_instruction(
        mybir.InstActivation(
            name=eng.bass.get_next_instruction_name(),
            func=func,
            ins=inputs,
            outs=[eng.lower_ap(out)],
        )
    )


def build_nc():
    # Bacc (not raw Bass): its compile() legalizes sync waits for walrus
    # (TRN2 allows at most one wait per instruction).
    nc = bacc.Bacc(None, target_bir_lowering=False, debug=False)

    x_d = nc.declare_dram_parameter("x", [T, D], FP32, isOutput=False)
    Wa_d = nc.declare_dram_parameter("Wa", [O, O], FP32, isOutput=False)
    Ua_d = nc.declare_dram_parameter("Ua", [D, O], FP32, isOutput=False)
    Uo_d = nc.declare_dram_parameter("Uo", [D, O], FP32, isOutput=False)
    Co_d = nc.declare_dram_parameter("Co", [D, O], FP32, isOutput=False)
    Va_d = nc.declare_dram_parameter("Va_col", [128, OT], FP32, isOutput=False)
    Ba_d = nc.declare_dram_parameter("Ba_col", [128, OT], FP32, isOutput=False)
    Bo_d = nc.declare_dram_parameter("Bo_col", [128, OT], FP32, isOutput=False)
    oeb_d = nc.declare_dram_parameter("oeb", [128, 2 * OT], FP32, isOutput=False)
    Id_d = nc.declare_dram_parameter("Id", [128, 128], FP32, isOutput=False)
    out_d = nc.declare_dram_parameter("out", [T, O], FP32, isOutput=True)

    with tile.TileContext(nc) as tc:
        with (
            tc.tile_pool(name="persist", bufs=1) as pp,
            tc.tile_pool(name="wpool", bufs=1) as wp,
        ):
            # ---- persistent SBUF tensors ----
            x_sb = pp.tile([128, D], FP32, tag="x")          # [tau, d]
            xT_sb = pp.tile([128, T * DT], FP32, tag="xT")   # tile dt at cols dt*128+tau
            Wa_sb = wp.tile([128, O * OT], FP32, tag="Wa")   # [o'', ot*O + o'] (0.5-scaled)
            Ua_sb = wp.tile([128, O * DT], FP32, tag="Ua")
            Uo_sb = wp.tile([128, O * DT], FP32, tag="Uo")
            Co_sb = wp.tile([128, O * DT], FP32, tag="Co")
            Va_sb = pp.tile([128, OT], FP32, tag="Va")
            Ba_sb = pp.tile([128, OT], FP32, tag="Ba")
            Bo_sb = pp.tile([128, OT], FP32, tag="Bo")
            oeb_sb = pp.tile([128, 2 * OT], FP32, tag="oeb")  # [1|embWo] pairs
            Id_sb = pp.tile([128, 128], FP32, tag="Id")
            UaH_sb = pp.tile([128, T * OT], FP32, tag="UaH")   # [o'', ot*T+tau]
            IUoB_sb = pp.tile([128, T * OT], FP32, tag="IUoB")  # [o'', tau*OT+kt]
            ICo_sb = pp.tile([128, O], FP32, tag="ICo")         # [tau, o]
            ones128 = pp.tile([128, 128], FP32, tag="ones128")
            ones_row = pp.tile([1, 128], FP32, tag="ones_r")
            zstate = pp.tile([128, OT], FP32, tag="zstate")
            out_sb = pp.tile([128, T * OT], FP32, tag="outb")  # [o'', t*OT+kt]

            # ---- DMA in (one strided DMA per weight; ordered by first use) ----
            def load_w(dst, src):
                # DRAM [512, O] -> SBUF [128, 4*O]: partition p <- row a*128+p
                nc.sync.dma_start(
                    dst[:, :].rearrange("p (a o) -> p a o", a=DT),
                    src.rearrange("(a p) o -> p a o", p=128),
                )

            nc.sync.dma_start(x_sb[:, :], x_d[:, :])
            nc.sync.dma_start(Id_sb[:, :], Id_d[:, :])
            load_w(Ua_sb, Ua_d)
            load_w(Wa_sb, Wa_d)
            load_w(Uo_sb, Uo_d)
            load_w(Co_sb, Co_d)
            nc.sync.dma_start(Va_sb[:, :], Va_d[:, :])
            nc.sync.dma_start(Ba_sb[:, :], Ba_d[:, :])
            nc.sync.dma_start(Bo_sb[:, :], Bo_d[:, :])
            nc.sync.dma_start(oeb_sb[:, :], oeb_d[:, :])

            # ---- constants ----
            nc.vector.memset(ones128[:, :], 1.0)
            nc.vector.memset(ones_row[:, :], 1.0)
            nc.vector.memset(zstate[:, :], 0.0)

            # ---- precompute ----
            with tc.tile_pool(name="pre_ps", bufs=2, space="PSUM") as prep:
                # xT: transpose x tiles
                for dt in range(DT):
                    pt = prep.tile([128, 128], FP32, tag="pt")
                    nc.tensor.transpose(
                        pt[:, :], x_sb[:, dt * 128:(dt + 1) * 128], Id_sb[:, :]
                    )
                    nc.vector.tensor_copy(xT_sb[:, dt * 128:(dt + 1) * 128], pt[:, :])
                # UaH_T[o'', ot*T+tau] = sum_d Ua[d, o] * x[tau, d]  (+Ba_adj)
                for ot in range(OT):
                    pu = prep.tile([128, 128], FP32, tag="pu")
                    for dt in range(DT):
                        nc.tensor.matmul(
                            pu[:, :],
                            Ua_sb[:, dt * O + ot * 128: dt * O + (ot + 1) * 128],
                            xT_sb[:, dt * 128:(dt + 1) * 128],
                            start=(dt == 0),
                            stop=(dt == DT - 1),
                        )
                    nc.scalar.activation(
                        UaH_sb[:, ot * T:(ot + 1) * T], pu[:, :], AF.Identity,
                        bias=Ba_sb[:, ot:ot + 1],
                    )
                # IUoB[o'', tau*OT+kt] = x[tau]@Uo + Bo
                for ot in range(OT):
                    pi = prep.tile([128, 128], FP32, tag="pu")
                    for dt in range(DT):
                        nc.tensor.matmul(
                            pi[:, :],
                            Uo_sb[:, dt * O + ot * 128: dt * O + (ot + 1) * 128],
                            xT_sb[:, dt * 128:(dt + 1) * 128],
                            start=(dt == 0),
                            stop=(dt == DT - 1),
                        )
                    dst = IUoB_sb[:, ot:ot + (T - 1) * OT + 1:OT]
                    nc.scalar.activation(
                        dst, pi[:, :], AF.Identity, bias=Bo_sb[:, ot:ot + 1]
                    )
                # ICo[tau, o] = x[tau] @ Co
                pc = prep.tile([128, O], FP32, tag="pc")
                for dt in range(DT):
                    nc.tensor.matmul(
                        pc[:, :],
                        xT_sb[:, dt * 128:(dt + 1) * 128],
                        Co_sb[:, dt * O:(dt + 1) * O],
                        start=(dt == 0),
                        stop=(dt == DT - 1),
                    )
                nc.vector.tensor_copy(ICo_sb[:, :], pc[:, :])

            # ---- the scan ----
            with (
                tc.tile_pool(name="sb_loop", bufs=3) as lp,
                tc.tile_pool(name="e_pool", bufs=2) as ep_pool,
                tc.tile_pool(name="was_ps", bufs=1, space="PSUM") as wasp,
                tc.tile_pool(name="pred_ps", bufs=2, space="PSUM") as predp,
                tc.tile_pool(name="sc_ps", bufs=1, space="PSUM") as scp,
                tc.tile_pool(name="misc_ps", bufs=1, space="PSUM") as miscp,
                tc.tile_pool(name="zb_ps", bufs=1, space="PSUM") as zbp,
            ):
              for _rep in range(REPEAT):
                for t in range(T):
                    state = zstate if t == 0 else out_sb[:, (t - 1) * OT: t * OT]
                    tm1 = (t - 1) % T

                    # --- ACT: th = tanh(0.5 * state)  (state = full pred) ---
                    th = lp.tile([128, OT], FP32, tag="th")
                    nc.scalar.activation(th[:, :], state, AF.Tanh, scale=0.5)

                    # --- DVE: ep = exp(state) = (1+th)/(1-th) ---
                    ep_a = lp.tile([128, OT], FP32, tag="epa")
                    ep_b = lp.tile([128, OT], FP32, tag="epb")
                    ep_rb = lp.tile([128, OT], FP32, tag="eprb")
                    ep = lp.tile([128, OT], FP32, tag="ep")
                    nc.vector.tensor_scalar_add(ep_a[:, :], th[:, :], 1.0)
                    nc.vector.tensor_scalar(
                        ep_b[:, :], th[:, :], -1.0, 1.0, ALU.mult, ALU.add
                    )
                    nc.vector.reciprocal(ep_rb[:, :], ep_b[:, :])
                    nc.vector.tensor_mul(ep[:, :], ep_a[:, :], ep_rb[:, :])

                    # --- PE: W1[o'] = sum_o th[o]*(0.5*Wa)[o,o'] -> psum pairs
                    # col 0 is written LAST so the e0 tanh carries the
                    # largest PE-sem wait and e1..e3's waits are subsumed
                    # (avoids the one-wait legalizer chaining the ladder
                    # through the ACT self-sem, which serializes it)
                    w_ps = wasp.tile([128, OT], FP32, tag="wps", name=f"wps_{t}")
                    for opt in reversed(range(OT)):
                        for ot in range(OT):
                            nc.tensor.matmul(
                                w_ps[:, opt:opt + 1],
                                Wa_sb[:, ot * O + opt * 128: ot * O + (opt + 1) * 128],
                                th[:, ot:ot + 1],
                                start=(ot == 0),
                                stop=(ot == OT - 1),
                            )

                    # --- PE: WoY numerators [Z2, numerW] = sum_o ep[o]*[1, embWo[o]]
                    misc = miscp.tile([128, 2], FP32, tag="misc", name=f"misc_{t}")
                    for kt in range(OT):
                        nc.tensor.matmul(
                            misc[0:1, 0:2],
                            ep[:, kt:kt + 1],
                            oeb_sb[:, 2 * kt:2 * kt + 2],
                            start=(kt == 0),
                            stop=(kt == OT - 1),
                        )

                    # --- DVE: stage [Z2, numerW] row to SBUF for the bcast mm
                    # (Pool/GPSIMD cannot touch PSUM on TRN2 per walrus)
                    zw_row = lp.tile([1, 2], FP32, tag="zwrow")
                    nc.vector.tensor_copy(zw_row[:, :], misc[0:1, 0:2])

                    # --- e-ladder bias staging ---
                    if PSUM_BIAS:
                        bias_of = lambda q: w_ps[:, q:q + 1]
                    else:
                        WaS_sb = lp.tile([128, OT], FP32, tag="WaS")
                        nc.vector.tensor_copy(WaS_sb[:, 0:2], w_ps[:, 0:2])
                        nc.vector.tensor_copy(WaS_sb[:, 2:4], w_ps[:, 2:4])
                        bias_of = lambda q: WaS_sb[:, q:q + 1]

                    # --- ACT: e_q = tanh(UaH_q + W1_q); PE: scores after each
                    e_sb = ep_pool.tile([128, O], FP32, tag="e")
                    sc = scp.tile([128, 1], FP32, tag="sc")
                    for q in range(OT):
                        _act_raw(
                            nc,
                            e_sb[:, q * T:(q + 1) * T],
                            UaH_sb[:, q * T:(q + 1) * T],
                            AF.Tanh,
                            bias=bias_of(q),
                        )
                    # --- PE: bcast [Z2, numerW] to all partitions ---
                    mb2 = miscp.tile([128, 2], FP32, tag="mb2", name=f"mb2_{t}")
                    nc.tensor.matmul(
                        mb2[:, 0:2], ones_row[:, :], zw_row[:, :],
                        start=True, stop=True,
                    )
                    for q in range(OT):
                        # scores[tau] += Va[o'] . e_T[o', tau]
                        nc.tensor.matmul(
                            sc[:, 0:1],
                            e_sb[:, q * T:(q + 1) * T],
                            Va_sb[:, q:q + 1],
                            start=(q == 0),
                            stop=(q == OT - 1),
                        )

                    # --- DVE: woyb = numerW / Z2, broadcast on all partitions
                    rz2b = lp.tile([128, 1], FP32, tag="rz2b")
                    woyb = lp.tile([128, 1], FP32, tag="woyb")
                    nc.vector.reciprocal(rz2b[:, :], mb2[:, 0:1])
                    nc.vector.tensor_mul(woyb[:, :], mb2[:, 1:2], rz2b[:, :])

                    # --- Pool: tmp = IUoB[t-1] + WoY ---
                    tmp = lp.tile([128, OT], FP32, tag="tmp")
                    nc.gpsimd.tensor_scalar_add(
                        tmp[:, :], IUoB_sb[:, tm1 * OT:(tm1 + 1) * OT], woyb[:, 0:1]
                    )

                    # --- ACT: softmax numerators over tau (scores are O(0.3))
                    eh = lp.tile([128, 1], FP32, tag="eh")
                    nc.scalar.activation(eh[:, :], sc[:, :], AF.Exp)

                    # --- PE: Z broadcast + u' = sum_tau eh * ICo[tau, o] ---
                    zb = zbp.tile([128, 1], FP32, tag="zb")
                    nc.tensor.matmul(
                        zb[:, :], ones128[:, :], eh[:, :], start=True, stop=True
                    )
                    pred = predp.tile([128, OT], FP32, tag="pred")
                    for m in range(OT):
                        nc.tensor.matmul(
                            pred[:, m:m + 1],
                            ICo_sb[:, m * 128:(m + 1) * 128],
                            eh[:, :],
                            start=True, stop=True,
                        )

                    # --- DVE: state_t = u'/Z + (IUoB[t-1] + WoY)  (full pred)
                    rz = lp.tile([128, 1], FP32, tag="rz")
                    nc.vector.reciprocal(rz[:, :], zb[:, :])
                    nc.vector.scalar_tensor_tensor(
                        out_sb[:, t * OT:(t + 1) * OT],
                        pred[:, :],
                        rz[:, 0:1],
                        tmp[:, :],
                        ALU.mult, ALU.add,
                    )

            # ---- epilogue: transpose to [tau, o] (out already has WoY) ----
            with (
                tc.tile_pool(name="ep_ps", bufs=2, space="PSUM") as epp,
                tc.tile_pool(name="ep_sb", bufs=2) as eps,
            ):
                outT = pp.tile([128, O], FP32, tag="outT")
                for kt in range(OT):
                    po = epp.tile([128, 128], FP32, tag="po")
                    nc.tensor.transpose(
                        po[:, :],
                        out_sb[:, kt:kt + (T - 1) * OT + 1:OT],
                        Id_sb[:, :],
                    )
                    nc.vector.tensor_copy(outT[:, kt * 128:(kt + 1) * 128], po[:, :])
                nc.sync.dma_start(out_d[:, :], outT[:, :])

    nc.compile()
    return nc


_NC_CACHE = {}


def _get_nc():
    if "nc" not in _NC_CACHE:
        _NC_CACHE["nc"] = build_nc()
    return _NC_CACHE["nc"]


def make_in_maps(inputs, Wa, Ua, Va, Ba, Wo, Uo, Co, Bo, emb):
    Wa = np.asarray(Wa, np.float32)
    Ua = np.asarray(Ua, np.float32)
    Uo = np.asarray(Uo, np.float32)
    Co = np.asarray(Co, np.float32)
    Va_col = np.ascontiguousarray(
        np.asarray(Va, np.float32)[:, 0].reshape(OT, 128).T
    )
    # fold sigmoid's affine (s = 0.5*tanh + 0.5) into the attention key bias:
    # WaS = s@Wa = 0.5*(tanh_h@Wa) + 0.5*colsum(Wa); the 0.5 factor on the
    # tanh term is folded into Wa itself (Wa_half below).
    ba_adj = (
        np.asarray(Ba, np.float64)[0]
        + 0.5 * np.asarray(Wa, np.float64).sum(axis=0)
    ).astype(np.float32)
    Wa_half = np.ascontiguousarray(0.5 * Wa)
    Ba_col = np.ascontiguousarray(ba_adj.reshape(OT, 128).T)
    Bo_col = np.ascontiguousarray(
        np.asarray(Bo, np.float32)[0].reshape(OT, 128).T
    )
    ebW = (np.asarray(emb, np.float64) @ np.asarray(Wo, np.float64)).astype(np.float32)
    ebW_col = ebW[:, 0].reshape(OT, 128).T
    oeb = np.ones((128, 2 * OT), dtype=np.float32)
    oeb[:, 1::2] = ebW_col
    oeb = np.ascontiguousarray(oeb)
    Id = np.eye(128, dtype=np.float32)
    shared = dict(
        Wa=Wa_half, Ua=Ua, Uo=Uo, Co=Co,
        Va_col=Va_col, Ba_col=Ba_col, Bo_col=Bo_col, oeb=oeb, Id=Id,
    )
    return [
        {"x": np.ascontiguousarray(np.asarray(inputs[b], np.float32)), **shared}
        for b in range(B)
    ]


def kernel(inputs, Wa, Ua, Va, Ba, Wo, Uo, Co, Bo, emb):
    nc = _get_nc()
    in_maps = make_in_maps(inputs, Wa, Ua, Va, Ba, Wo, Uo, Co, Bo, emb)
    res = run_bass_kernel_spmd(nc, in_maps, list(range(B)))
    out = np.stack([res.results[b]["out"] for b in range(B)], axis=0)
    return out.astype(np.float32)


if __name__ == "__main__":
    rng = np.random.default_rng(0)
    w = 0.02
    ins = dict(
        inputs=rng.standard_normal((B, T, D), dtype=np.float32),
        Wa=rng.standard_normal((O, O), dtype=np.float32) * w,
        Ua=rng.standard_normal((D, O), dtype=np.float32) * w,
        Va=rng.standard_normal((O, 1), dtype=np.float32) * w,
        Ba=rng.standard_normal((1, O), dtype=np.float32) * w,
        Wo=rng.standard_normal((O, 1), dtype=np.float32) * w,
        Uo=rng.standard_normal((D, O), dtype=np.float32) * w,
        Co=rng.standard_normal((D, O), dtype=np.float32) * w,
        Bo=rng.standard_normal((1, O), dtype=np.float32) * w,
        emb=rng.standard_normal((O, O), dtype=np.float32) * w,
    )
    out = kernel(**ins)
    print(out.shape, out.dtype, np.abs(out).mean())
